# revision 6
# baseline (speedup 1.0000x reference)
"""ChannelDiffusion kernel for 8 Trainium2 NeuronCores.

Reference computation (B=2, N=8192, D=1024, H=16, dh=64):
    qk = (x @ W_qk)  -> per (b,h): Gram matrix dot[c,d] = sum_n qk[h,c,n]qk[h,d,n]
    logits = (2*dot - q2[c] - q2[d]) / sqrt(N) * tau[h];  attn = softmax(logits)
    w = attn @ v;  out = w^T @ W_out

Sharding: core c handles batch c//4, tokens [(c%4)*2048, +2048).  Weights are
replicated.  The (16,64,64) Gram partials are AllReduce'd within each group of
4 cores (one group per batch element).

Key tricks:
  - float32r matmuls (4x faster than fp32 on the PE, ~1.5e-4 rel err).
  - logits are symmetric and <= 0, so softmax needs no max subtraction and the
    *unnormalized* exp(logits) matrix E is symmetric: it can be used directly
    as the matmul stationary operand (lhsT) without a transpose; the 1/Z
    normalization is applied per-partition when copying PSUM out.
  - q2 = diag(dot) extracted with a diagonal mask + free-axis reduce; the
    "q2[d] along the free axis" broadcast is a block-diag-ones matmul.
  - heads are processed in pairs (dh=64 -> 128 partitions) via block-diagonal
    stationary operands so every matmul has K=128.
"""
import math

import numpy as np

import concourse.bass as bass
import concourse.mybir as mybir
import concourse.tile as tile
from concourse import bacc
from concourse.bass_utils import run_bass_kernel_spmd
from concourse.masks import make_identity

P = 128
B, N, D, H = 2, 8192, 1024, 16
DH = D // H          # 64
CORES = 8
GROUPS = [[0, 1, 2, 3], [4, 5, 6, 7]]
T = (B * N) // CORES          # 2048 tokens per core
TCH = T // P                  # 16 token chunks of 128
KC = D // P                   # 8 contraction chunks
HP = H // 2                   # 8 head pairs
SQRT_N_INV = 1.0 / math.sqrt(N)

F32 = mybir.dt.float32
F32R = mybir.dt.float32r
X = mybir.AxisListType.X
Alu = mybir.AluOpType
Act = mybir.ActivationFunctionType


def build_kernel() -> bacc.Bacc:
    nc = bacc.Bacc("TRN2", target_bir_lowering=False, debug=False,
                   num_devices=CORES)

    x_d = nc.dram_tensor("x", [T, D], F32, kind="ExternalInput")
    wqk_d = nc.dram_tensor("W_qk", [D, D], F32R, kind="ExternalInput")
    wv_d = nc.dram_tensor("W_v", [D, D], F32R, kind="ExternalInput")
    wout_d = nc.dram_tensor("W_out", [D, D], F32R, kind="ExternalInput")
    tau_d = nc.dram_tensor("tau", [H], F32, kind="ExternalInput")
    out_d = nc.dram_tensor("out", [T, D], F32, kind="ExternalOutput")

    with tile.TileContext(nc) as tc:
        _emit(nc, tc, x_d, wqk_d, wv_d, wout_d, tau_d, out_d)
    nc.compile()
    return nc


def _emit(nc, tc, x_d, wqk_d, wv_d, wout_d, tau_d, out_d):
    from contextlib import ExitStack

    outer = ExitStack()
    with outer:
        small = outer.enter_context(tc.tile_pool(name="small", bufs=1))
        dram = outer.enter_context(tc.tile_pool(name="dram", bufs=1, space="DRAM"))

        # ---------------- prologue: constants ----------------
        ident = small.tile([P, P], F32, name="ident")
        make_identity(nc, ident[:])

        # dmask[p, hp, d] = 1.0 iff d == p % 64   (diagonal of each head block)
        pv = small.tile([P, 1], F32, name="pv")
        nc.gpsimd.iota(pv[:], pattern=[[0, 1]], base=0, channel_multiplier=1,
                       allow_small_or_imprecise_dtypes=True)
        ge = small.tile([P, 1], F32, name="ge")
        nc.vector.tensor_scalar(ge[:], pv[:], 64.0, None, op0=Alu.is_ge)
        nc.vector.tensor_scalar_mul(ge[:], ge[:], 64.0)
        nc.vector.tensor_sub(pv[:], pv[:], ge[:])          # pv = p mod 64
        dv = small.tile([P, HP, DH], F32, name="dv")
        nc.gpsimd.iota(dv[:], pattern=[[0, HP], [1, DH]], base=0,
                       channel_multiplier=0, allow_small_or_imprecise_dtypes=True)
        dmask = small.tile([P, HP, DH], F32, name="dmask")
        nc.vector.tensor_tensor(dmask[:], dv[:],
                                pv[:, :, None].to_broadcast([P, HP, DH]),
                                Alu.is_equal)

        # BD1: block-diagonal ones [128,128] (64x64 blocks).
        # (memset can't write f32r; build with tensor_scalar from the f32 ident)
        bd1 = small.tile([P, P], F32R, name="bd1")
        nc.vector.tensor_scalar_mul(bd1[:], ident[:], 0.0)
        nc.vector.tensor_scalar(bd1[0:64, 0:64], ident[0:64, 0:64],
                                0.0, 1.0, op0=Alu.mult, op1=Alu.add)
        nc.vector.tensor_scalar(bd1[64:128, 64:128], ident[64:128, 64:128],
                                0.0, 1.0, op0=Alu.mult, op1=Alu.add)

        # taum[p, hp] = tau[2*hp + (p >= 64)] / sqrt(N)
        tau16 = small.tile([H, 1], F32, name="tau16")
        nc.sync.dma_start(tau16[:], tau_d[:, None])
        pv16 = small.tile([H, 1], F32, name="pv16")
        nc.gpsimd.iota(pv16[:], pattern=[[0, 1]], base=0, channel_multiplier=1,
                       allow_small_or_imprecise_dtypes=True)
        dv16 = small.tile([H, H], F32, name="dv16")
        nc.gpsimd.iota(dv16[:], pattern=[[1, H]], base=0, channel_multiplier=0,
                       allow_small_or_imprecise_dtypes=True)
        taud = small.tile([H, H], F32R, name="taud")
        nc.vector.tensor_tensor(taud[:], dv16[:],
                                pv16[:, 0:1].to_broadcast([H, H]), Alu.is_equal)
        nc.vector.tensor_tensor(taud[:], taud.bitcast(F32)[:],
                                tau16[:, 0:1].to_broadcast([H, H]), Alu.mult)
        ones16 = small.tile([H, P], F32R, name="ones16")
        nc.vector.tensor_scalar(ones16[:], ident[0:H, :], 0.0, 1.0,
                                op0=Alu.mult, op1=Alu.add)
        taum = small.tile([P, HP], F32, name="taum")
        with tc.tile_pool(name="psum_pro", bufs=1, space="PSUM") as psum_pro:
            tb_ps = psum_pro.tile([P, H], F32, name="tb_ps")
            nc.tensor.matmul(tb_ps[:], ones16[:], taud[:], start=True, stop=True)
            nc.scalar.activation(taum[0:64, :], tb_ps[0:64, 0:H:2], Act.Copy,
                                 scale=SQRT_N_INV)
            nc.scalar.activation(taum[64:128, :], tb_ps[64:128, 1:H:2], Act.Copy,
                                 scale=SQRT_N_INV)

        # collective buffers
        cc_in = dram.tile([H, DH, DH], F32, name="cc_in")
        cc_out = dram.tile([H, DH, DH], F32, name="cc_out")

        # x^T, alive through stage C (left stack)
        xT_ctx = ExitStack()
        pool_xT = xT_ctx.enter_context(tc.tile_pool(name="xT", bufs=1))
        xT = pool_xT.tile([P, KC, T], F32R, name="xT")

        # W_v above x^T on the left stack (released right after stage C)
        pool_wv_ctx = ExitStack()
        pool_wv = pool_wv_ctx.enter_context(tc.tile_pool(name="wv", bufs=1))
        wv = pool_wv.tile([P, KC, D], F32R, name="wv")
        for k in range(KC):
            nc.sync.dma_start(wv[:, k, :], wv_d[k * P:(k + 1) * P, :])

        partial = small.tile([P, HP, DH], F32, name="partial")

        # ---------------- stage A+B: load/transpose x, qk proj, Gram ----------
        with ExitStack() as ab:
            pool_wqk = ab.enter_context(tc.tile_pool(name="wqk", bufs=1))
            wqk = pool_wqk.tile([P, KC, D], F32R, name="wqk")
            for k in range(KC):
                nc.sync.dma_start(wqk[:, k, :], wqk_d[k * P:(k + 1) * P, :])

            pool_xa = ab.enter_context(tc.tile_pool(name="xa", bufs=3))
            pool_qk = ab.enter_context(tc.tile_pool(name="qk", bufs=3))
            psum_tr = ab.enter_context(
                tc.tile_pool(name="psum_tr", bufs=2, space="PSUM"))
            psum_qk = ab.enter_context(
                tc.tile_pool(name="psum_qk", bufs=2, space="PSUM"))
            psum_gr = ab.enter_context(
                tc.tile_pool(name="psum_gr", bufs=4, space="PSUM"))

            # Gram accumulators: 4 banks, two [128,256] regions each
            gram = [psum_gr.tile([P, 512], F32, name=f"gram{g}", tag="gram") for g in range(4)]

            for t in range(TCH):
                xa = pool_xa.tile([P, D], F32, name="xa")
                nc.sync.dma_start(xa[:], x_d[t * P:(t + 1) * P, :])
                for k in range(KC):
                    ptr = psum_tr.tile([P, P], F32, name="ptr")
                    nc.tensor.transpose(ptr[:], xa[:, k * P:(k + 1) * P], ident[:])
                    eng = nc.vector.tensor_copy if k % 2 == 0 else nc.scalar.copy
                    eng(xT[:, k, t * P:(t + 1) * P], ptr[:])

                pq = [psum_qk.tile([P, 512], F32, name=f"pq{no}", tag="pq") for no in range(2)]
                for no in range(2):
                    for k in range(KC):
                        nc.tensor.matmul(pq[no][:], xT[:, k, t * P:(t + 1) * P],
                                         wqk[:, k, no * 512:(no + 1) * 512],
                                         start=(k == 0), stop=(k == KC - 1))
                qk_m = pool_qk.tile([P, D], F32R, name="qk_m")
                nc.scalar.copy(qk_m[:, 0:512], pq[0][:])
                nc.vector.tensor_copy(qk_m[:, 512:1024], pq[1][:])

                for hp in range(HP):
                    g, half = hp // 2, hp % 2
                    nc.tensor.matmul(
                        gram[g][:, half * 256:(half + 1) * 256],
                        qk_m[:, hp * P:(hp + 1) * P],
                        qk_m[:, (hp // 2) * 256:(hp // 2) * 256 + 256],
                        start=(t == 0), stop=(t == TCH - 1),
                        skip_group_check=True)

            # extract per-head partial Gram blocks -> [128(parity,c), hp, d]
            for hp in range(HP):
                g, half = hp // 2, hp % 2
                off = half * 256 + half * 128
                nc.vector.tensor_copy(partial[0:64, hp, :],
                                      gram[g][0:64, off:off + 64])
                nc.vector.tensor_copy(partial[64:128, hp, :],
                                      gram[g][64:128, off + 64:off + 128])

            # partial -> cc_in[h, c, d]  (h = 2*hp + parity)
            nc.sync.dma_start(cc_in[0:H:2].rearrange("h c d -> c h d"),
                              partial[0:64, :, :])
            nc.sync.dma_start(cc_in[1:H:2].rearrange("h c d -> c h d"),
                              partial[64:128, :, :])
            nc.gpsimd.collective_compute(
                "AllReduce", Alu.add, replica_groups=GROUPS,
                ins=[cc_in.opt()], outs=[cc_out.opt()])

        # ---------------- stage C: v projection (channel-major) --------------
        # vB lives on the right stack: C..E, overlapping xT/wv release
        vB_ctx = ExitStack()
        pool_vB = vB_ctx.enter_context(tc.tile_pool(name="vB", bufs=1, side="right"))
        vB = pool_vB.tile([P, KC, T], F32R, name="vB")
        with tc.tile_pool(name="psum_v", bufs=4, space="PSUM") as psum_v:
            for o in range(KC):
                for s in range(T // 512):
                    pv_ = psum_v.tile([P, 512], F32, name="pv_")
                    for k in range(KC):
                        nc.tensor.matmul(pv_[:], wv[:, k, o * P:(o + 1) * P],
                                         xT[:, k, s * 512:(s + 1) * 512],
                                         start=(k == 0), stop=(k == KC - 1))
                    eng = nc.vector.tensor_copy if (o + s) % 2 == 0 else nc.scalar.copy
                    eng(vB[:, o, s * 512:(s + 1) * 512], pv_[:])
        pool_wv_ctx.close()
        xT_ctx.close()

        # W_out load (overlaps stages D/E)
        wout_ctx = ExitStack()
        pool_wout = wout_ctx.enter_context(tc.tile_pool(name="wout", bufs=1))
        wout = pool_wout.tile([P, KC, D], F32R, name="wout")
        for k in range(KC):
            nc.sync.dma_start(wout[:, k, :], wout_d[k * P:(k + 1) * P, :])

        # ---------------- stage D: attention weights ----------------
        dot_sb = small.tile([P, HP, DH], F32, name="dot_sb")
        nc.sync.dma_start(dot_sb[0:64, :, :],
                          cc_out[0:H:2].rearrange("h c d -> c h d"))
        nc.sync.dma_start(dot_sb[64:128, :, :],
                          cc_out[1:H:2].rearrange("h c d -> c h d"))

        masked = small.tile([P, HP, DH], F32R, name="masked")
        nc.vector.tensor_mul(masked[:], dot_sb[:], dmask[:])
        q2 = small.tile([P, HP], F32, name="q2")
        nc.vector.reduce_sum(q2[:], masked.bitcast(F32)[:], axis=X)

        lg = small.tile([P, HP, DH], F32, name="lg")
        e_sb = small.tile([P, HP, DH], F32R, name="e_sb")
        z_sum = small.tile([P, HP], F32, name="z_sum")
        zinv = small.tile([P, HP], F32, name="zinv")
        with tc.tile_pool(name="psum_d", bufs=1, space="PSUM") as psum_d:
            q2d = psum_d.tile([P, HP, DH], F32, name="q2d")
            nc.tensor.matmul(q2d[:], bd1[:], masked[:], start=True, stop=True)
            nc.vector.tensor_scalar_mul(lg[:], dot_sb[:], 2.0)
            nc.vector.tensor_sub(lg[:], lg[:],
                                 q2[:, :, None].to_broadcast([P, HP, DH]))
            nc.vector.tensor_sub(lg[:], lg[:], q2d[:])
            nc.vector.tensor_mul(lg[:], lg[:],
                                 taum[:, :, None].to_broadcast([P, HP, DH]))
        nc.scalar.activation(e_sb[:], lg[:], Act.Exp)
        nc.vector.reduce_sum(z_sum[:], e_sb.bitcast(F32)[:], axis=X)
        nc.vector.reciprocal(zinv[:], z_sum[:])

        # ---------------- stage E: w = attn @ v ----------------
        wB_ctx = ExitStack()
        pool_wB = wB_ctx.enter_context(tc.tile_pool(name="wB", bufs=1))
        wB = pool_wB.tile([P, KC, T], F32R, name="wB")
        with tc.tile_pool(name="bd", bufs=2) as pool_bd, \
             tc.tile_pool(name="psum_w", bufs=4, space="PSUM") as psum_w:
            for hp in range(HP):
                bd = pool_bd.tile([P, P], F32R, name="bd")
                nc.vector.tensor_scalar_mul(bd[:], ident[:], 0.0)
                nc.vector.tensor_copy(bd[0:64, 0:64], e_sb[0:64, hp, :])
                nc.vector.tensor_copy(bd[64:128, 64:128], e_sb[64:128, hp, :])
                for nt in range(T // 512):
                    pw = psum_w.tile([P, 512], F32, name="pw")
                    nc.tensor.matmul(pw[:], bd[:],
                                     vB[:, hp, nt * 512:(nt + 1) * 512],
                                     start=True, stop=True)
                    nc.scalar.activation(wB[:, hp, nt * 512:(nt + 1) * 512],
                                         pw[:], Act.Copy,
                                         scale=zinv[:, hp:hp + 1])
        vB_ctx.close()

        # ---------------- stage F: out = w^T @ W_out ----------------
        with tc.tile_pool(name="outp", bufs=2) as pool_out, \
             tc.tile_pool(name="psum_o", bufs=4, space="PSUM") as psum_o:
            for mt in range(TCH):
                po = [psum_o.tile([P, 512], F32, name=f"po{no}", tag="po") for no in range(2)]
                for no in range(2):
                    for k in range(KC):
                        nc.tensor.matmul(po[no][:], wB[:, k, mt * P:(mt + 1) * P],
                                         wout[:, k, no * 512:(no + 1) * 512],
                                         start=(k == 0), stop=(k == KC - 1))
                ot = pool_out.tile([P, D], F32, name="ot")
                nc.scalar.copy(ot[:, 0:512], po[0][:])
                nc.vector.tensor_copy(ot[:, 512:1024], po[1][:])
                nc.sync.dma_start(out_d[mt * P:(mt + 1) * P, :], ot[:])
        wB_ctx.close()
        wout_ctx.close()


_NC_CACHE = None


def _get_nc():
    global _NC_CACHE
    if _NC_CACHE is None:
        _NC_CACHE = build_kernel()
    return _NC_CACHE


def kernel(**inputs) -> np.ndarray:
    x = np.ascontiguousarray(np.asarray(inputs["x"], dtype=np.float32))
    w_qk = np.ascontiguousarray(np.asarray(inputs["W_qk"], dtype=np.float32))
    w_v = np.ascontiguousarray(np.asarray(inputs["W_v"], dtype=np.float32))
    w_out = np.ascontiguousarray(np.asarray(inputs["W_out"], dtype=np.float32))
    tau = np.ascontiguousarray(
        np.asarray(inputs["tau"], dtype=np.float32).reshape(H))

    nc = _get_nc()
    in_maps = []
    for c in range(CORES):
        b, s = c // 4, c % 4
        in_maps.append({
            "x": np.ascontiguousarray(x[b, s * T:(s + 1) * T, :]),
            "W_qk": w_qk, "W_v": w_v, "W_out": w_out, "tau": tau,
        })
    res = run_bass_kernel_spmd(nc, in_maps, core_ids=list(range(CORES)))
    out = np.empty((B, N, D), dtype=np.float32)
    for c in range(CORES):
        b, s = c // 4, c % 4
        out[b, s * T:(s + 1) * T, :] = res.results[c]["out"]
    return out
